# revision 16
# baseline (speedup 1.0000x reference)
"""AttentionBlock (GroupNorm + QKV + 8-head spatial attention + proj + residual)
on 8 Trainium2 NeuronCores.

Sharding: 16 head-batches (B=2 x NH=8) are split 2-per-core; cores 0-3 take
batch 0, cores 4-7 batch 1.  Each core:
  - loads its batch's x [512, 4096] (bf16), GroupNorm statistics on-chip via
    bn_stats tracking the DMA chunk by chunk; warm-up matmuls keep the PE
    p-state high through the load,
  - folds the GroupNorm affine into the QKV weights (W' = W*A per channel,
    bias' = W@B + qkv_b) so x feeds the QKV matmuls directly,
  - computes q/k/v in [c, L] layout; v is DMA-transposed (hardware XBAR)
    into vT [s, c] chunks so the PE never burns cycles transposing; the
    softmax-denominator ones-columns are memset once,
  - scores are computed in [s, t] layout with K=64 partition-sliced matmuls;
    exp (ACT) is the bottleneck and runs back-to-back; the a_plus
    accumulation lags one j so the in-order PE stream never stalls,
  - the projection of each finished t-stripe goes to its own a_cat tile
    (no false deps) and is spread one [128,512] unit at a time over the
    next h==1 stripe's j-loop; partials stream out over DMA in bf16.
Host sums the 4 partials per batch, adds proj_b and the residual.

All big matmuls run in bf16 (PSUM accumulation stays fp32).  Measured
accuracy ~3.5e-4 rel vs the 2e-2 gate.  Steady state is ACT(exp)-bound:
256 exps of [128,1024] back-to-back at ~1.1us each.
"""

import numpy as np
import ml_dtypes

import concourse.bacc as bacc
import concourse.tile as tile
from concourse import mybir
from concourse.bass_utils import run_bass_kernel_spmd

B, C = 2, 512
L = 64 * 64           # 4096
NH = 8                # heads total
CH = 64               # channels per head
G = 32                # groups
EPS = 1e-5
N_CORES = 8
HEADS_PER_CORE = 2

F32 = mybir.dt.float32
F32R = mybir.dt.float32r
BF16 = mybir.dt.bfloat16
I32 = mybir.dt.int32
I16 = mybir.dt.int16
AF = mybir.ActivationFunctionType
ALU = mybir.AluOpType

TSUP = 1024           # t-stripe width (2 PSUM banks)
NT = L // TSUP        # 4 stripes
SJ = 32               # number of 128-wide s-chunks

_PROGRAM = None


def build_program():
    nc = bacc.Bacc()
    xb = nc.declare_dram_parameter("xb", [128, 4, L], BF16, isOutput=False).ap()
    gmask = nc.declare_dram_parameter("gmask", [128, 4, G], F32R, isOutput=False).ap()
    bmask = nc.declare_dram_parameter("bmask", [G, 4, 128], F32R, isOutput=False).ap()
    gamma4 = nc.declare_dram_parameter("gamma4", [4, 128], F32, isOutput=False).ap()
    beta4 = nc.declare_dram_parameter("beta4", [4, 128], F32, isOutput=False).ap()
    wqT = nc.declare_dram_parameter("wqT", [C, 128], BF16, isOutput=False).ap()
    wkT = nc.declare_dram_parameter("wkT", [C, 128], BF16, isOutput=False).ap()
    wvT = nc.declare_dram_parameter("wvT", [C, 128], BF16, isOutput=False).ap()
    qb = nc.declare_dram_parameter("qb", [128], F32, isOutput=False).ap()
    kb = nc.declare_dram_parameter("kb", [128], F32, isOutput=False).ap()
    vb = nc.declare_dram_parameter("vb", [128], F32, isOutput=False).ap()
    pwT = nc.declare_dram_parameter("pwT", [128, C], BF16, isOutput=False).ap()
    part = nc.declare_dram_parameter("part", [C, L], BF16, isOutput=True).ap()

    with tile.TileContext(nc) as tc:
        with (
            tc.tile_pool(name="consts", bufs=1) as consts,
            tc.tile_pool(name="big", bufs=1) as big,
            tc.tile_pool(name="work", bufs=2) as work,
            tc.tile_pool(name="ps", bufs=1, space="PSUM") as ps,
        ):
            # ---- small consts needed for GroupNorm stats first ----
            sb_gmask = consts.tile([128, 4, G], F32R)
            nc.sync.dma_start(out=sb_gmask, in_=gmask)
            sb_bmask = consts.tile([G, 4, 128], F32R)
            nc.sync.dma_start(out=sb_bmask, in_=bmask)
            sb_gamma = consts.tile([128, 4], F32)
            nc.sync.dma_start(out=sb_gamma, in_=gamma4.rearrange("t p -> p t"))
            sb_beta = consts.tile([128, 4], F32)
            nc.sync.dma_start(out=sb_beta, in_=beta4.rearrange("t p -> p t"))
            eps32 = consts.tile([32, 1], F32)
            nc.vector.memset(eps32, EPS)
            m1c = consts.tile([1, 1], I32)
            nc.vector.memset(m1c, -1)
            dummy_w = consts.tile([128, 128], BF16)
            nc.vector.memset(dummy_w, 0.0)

            # ---- load x; bn_stats tracks the DMA; PE warm-up in parallel ----
            xt = big.tile([128, 4, L], BF16)
            stats = work.tile([128, 4, 8, 6], F32, bufs=1)
            for s in range(8):
                ns = slice(s * 512, (s + 1) * 512)
                nc.sync.dma_start(out=xt[:, :, ns], in_=xb[:, :, ns])
                for t in range(4):
                    nc.vector.bn_stats(out=stats[:, t, s, :], in_=xt[:, t, ns])
                if s >= 1:
                    # p-state warm-up: harmless matmuls paced by the DMA so the
                    # PE clock is at full speed when the QKV burst starts
                    for _ in range(6):
                        wps = ps.tile([128, 512], F32, tag="pp", bufs=2, name="wps")
                        nc.tensor.matmul(wps, dummy_w, xt[:, 0, ns],
                                         start=True, stop=True)
            for _ in range(10):
                wps = ps.tile([128, 512], F32, tag="pp", bufs=2, name="wps")
                nc.tensor.matmul(wps, dummy_w, xt[:, 0, 3584:4096],
                                 start=True, stop=True)
            mv = work.tile([128, 4, 2], F32, bufs=1)
            for t in range(4):
                nc.vector.bn_aggr(out=mv[:, t, :], in_=stats[:, t, :, :])
            # per-channel [mean, var+mean^2]
            stats2 = work.tile([128, 4, 2], F32R, bufs=1)
            msq = work.tile([128, 4, 1], F32, bufs=1)
            nc.vector.tensor_copy(out=stats2[:, :, 0:1], in_=mv[:, :, 0:1])
            nc.vector.tensor_mul(msq, mv[:, :, 0:1], mv[:, :, 0:1])
            nc.vector.tensor_add(stats2[:, :, 1:2], mv[:, :, 1:2], msq)
            # group stats via mask matmul: [32, 2] = (mean_g, E[x^2]_g)
            gps = ps.tile([32, 2], F32, tag="apl0")
            for t in range(4):
                nc.tensor.matmul(
                    gps, sb_gmask[:, t, :], stats2[:, t, :],
                    start=(t == 0), stop=(t == 3),
                )
            gs = work.tile([32, 2], F32, bufs=1)
            nc.vector.tensor_copy(out=gs, in_=gps)
            msqg = work.tile([32, 1], F32, bufs=1)
            varg = work.tile([32, 1], F32, bufs=1)
            nc.vector.tensor_mul(msqg, gs[:, 0:1], gs[:, 0:1])
            nc.vector.tensor_sub(varg, gs[:, 1:2], msqg)
            # rstd = exp(-0.5*ln(var+eps))  (Ln+Exp share one ACT table set)
            lng = work.tile([32, 1], F32, bufs=1)
            nc.scalar.activation(out=lng, in_=varg, func=AF.Ln, bias=eps32, scale=1.0)
            rstdg = work.tile([32, 1], F32, bufs=1)
            nc.scalar.activation(out=rstdg, in_=lng, func=AF.Exp, scale=-0.5)
            gstats2 = work.tile([32, 2], F32R, bufs=1)
            nc.vector.tensor_copy(out=gstats2[:, 0:1], in_=gs[:, 0:1])
            nc.vector.tensor_copy(out=gstats2[:, 1:2], in_=rstdg)

            # ---- weights (arrive during/after x) ----
            sb_wq = consts.tile([128, 4, 128], BF16)
            nc.sync.dma_start(out=sb_wq, in_=wqT.rearrange("(kk p) m -> p kk m", p=128))
            sb_wk = consts.tile([128, 4, 128], BF16)
            nc.sync.dma_start(out=sb_wk, in_=wkT.rearrange("(kk p) m -> p kk m", p=128))
            sb_wv = consts.tile([128, 4, 128], BF16)
            nc.sync.dma_start(out=sb_wv, in_=wvT.rearrange("(kk p) m -> p kk m", p=128))
            sb_pw = consts.tile([128, C], BF16)
            nc.sync.dma_start(out=sb_pw, in_=pwT)
            sb_qb = consts.tile([128, 1], F32)
            nc.sync.dma_start(out=sb_qb, in_=qb.unsqueeze(1))
            sb_kb = consts.tile([128, 1], F32)
            nc.sync.dma_start(out=sb_kb, in_=kb.unsqueeze(1))
            sb_vb = consts.tile([128, 1], F32)
            nc.sync.dma_start(out=sb_vb, in_=vb.unsqueeze(1))

            # ---- per-channel affine A, Bs  (hid = x*A + Bs) ----
            A_all = work.tile([128, 4], F32, bufs=1)
            Bcol = work.tile([128, 4, 2], BF16, bufs=1)
            for t in range(4):
                cst = ps.tile([128, 2], F32, tag="apl1")
                nc.tensor.matmul(
                    cst, sb_bmask[:, t, :], gstats2, start=True, stop=True
                )
                nc.vector.tensor_mul(A_all[:, t:t + 1], cst[:, 1:2], sb_gamma[:, t:t + 1])
                tmp = work.tile([128, 1], F32, tag="tmp")
                nc.vector.tensor_mul(tmp, cst[:, 0:1], A_all[:, t:t + 1])
                nc.vector.tensor_sub(Bcol[:, t, :], sb_beta[:, t:t + 1].broadcast_to([128, 2]), tmp.broadcast_to([128, 2]))

            # ---- fold affine into QKV weights ----
            # bias' = W^T @ Bs + b first (reads original W), then W *= A in place
            cq_ps = ps.tile([128, 2], F32, tag="sc", bufs=2)
            ck_ps = ps.tile([128, 2], F32, tag="apl0")
            cv_ps = ps.tile([128, 2], F32, tag="apl1")
            for t in range(4):
                nc.tensor.matmul(cq_ps, sb_wq[:, t, :], Bcol[:, t, :],
                                 start=(t == 0), stop=(t == 3))
                nc.tensor.matmul(ck_ps, sb_wk[:, t, :], Bcol[:, t, :],
                                 start=(t == 0), stop=(t == 3))
                nc.tensor.matmul(cv_ps, sb_wv[:, t, :], Bcol[:, t, :],
                                 start=(t == 0), stop=(t == 3))
            qc = consts.tile([128, 1], F32)
            nc.vector.tensor_add(qc, cq_ps[:, 0:1], sb_qb)
            kc = consts.tile([128, 1], F32)
            nc.vector.tensor_add(kc, ck_ps[:, 0:1], sb_kb)
            vc_b = consts.tile([128, 1], F32)
            nc.vector.tensor_add(vc_b, cv_ps[:, 0:1], sb_vb)
            for t in range(4):
                nc.vector.tensor_scalar_mul(
                    out=sb_wq[:, t, :], in0=sb_wq[:, t, :], scalar1=A_all[:, t:t + 1])
                nc.vector.tensor_scalar_mul(
                    out=sb_wk[:, t, :], in0=sb_wk[:, t, :], scalar1=A_all[:, t:t + 1])
                nc.vector.tensor_scalar_mul(
                    out=sb_wv[:, t, :], in0=sb_wv[:, t, :], scalar1=A_all[:, t:t + 1])

            for _ in range(4):
                wps = ps.tile([128, 512], F32, tag="pp", bufs=2, name="wps")
                nc.tensor.matmul(wps, dummy_w, xt[:, 0, 3584:4096],
                                 start=True, stop=True)
            # ---- QKV in [c, L] layout; vT via hardware XBAR transpose ----
            q2 = big.tile([128, L], BF16)
            k2 = big.tile([128, L], BF16)
            vc2 = big.tile([128, L], BF16)
            # vT: [s, c] both heads + ones cols at 64 (h0) / 129 (h1)
            vt = big.tile([128, SJ, 160], BF16)
            nc.vector.memset(vt[:, :, 64:65], 1.0)
            nc.vector.memset(vt[:, :, 144:145], 1.0)

            def emit_q(n):
                ns = slice(n * 512, (n + 1) * 512)
                qp = ps.tile([128, 512], F32, tag="pp", bufs=2, name="qp")
                for kk in range(4):
                    nc.tensor.matmul(qp, sb_wq[:, kk, :], xt[:, kk, ns],
                                     start=(kk == 0), stop=(kk == 3))
                nc.vector.tensor_scalar_add(out=q2[:, ns], in0=qp, scalar1=qc)

            def emit_v(n):
                ns = slice(n * 512, (n + 1) * 512)
                vp = ps.tile([128, 512], F32, tag="pp", bufs=2, name="vp")
                for kk in range(4):
                    nc.tensor.matmul(vp, sb_wv[:, kk, :], xt[:, kk, ns],
                                     start=(kk == 0), stop=(kk == 3))
                nc.vector.tensor_scalar_add(out=vc2[:, ns], in0=vp, scalar1=vc_b)
                # XBAR-transpose this 512-col span into 4 vt chunks per head
                cs = slice(4 * n, 4 * n + 4)
                nc.sync.dma_start_transpose(out=vt[:, cs, 0:64], in_=vc2[0:64, ns])
                nc.sync.dma_start_transpose(out=vt[:, cs, 80:144], in_=vc2[64:128, ns])

            for n in range(8):
                ns = slice(n * 512, (n + 1) * 512)
                kp = ps.tile([128, 512], F32, tag="pp", bufs=2, name="kp")
                for kk in range(4):
                    nc.tensor.matmul(kp, sb_wk[:, kk, :], xt[:, kk, ns],
                                     start=(kk == 0), stop=(kk == 3))
                nc.vector.tensor_scalar_add(out=k2[:, ns], in0=kp, scalar1=kc)
            emit_q(0)
            emit_q(1)
            emit_v(0)
            emit_v(1)

            # ---- attention ----
            # Per (h, tsup) stripe of 1024 t-columns.  exp (ACT) is the
            # bottleneck and runs back-to-back; the a_plus accumulation lags
            # one j.  Remaining q/v chunks stream into stripe (0,0)'s slack.
            a_cats = [big.tile([128, TSUP], BF16, name=f"a_cat{i}")
                      for i in range(NT)]

            def recip_neg(den, width):
                """z = -1/den at ~18 bits via NOT-seed + 2 NR steps, all as
                plain DVE ops (the scheduler models these accurately, unlike
                the 8-pass InstReciprocal)."""
                nxz = work.tile([1, width], I32, tag="nx", name="nxz")
                nc.vector.tensor_scalar(out=nxz, in0=den.bitcast(I32),
                                        scalar1=m1c, scalar2=None,
                                        op0=ALU.bitwise_xor)
                # partition-0 copy of den (den may live on partition 64):
                # den0 = NOT(NOT(den)) — tensor_scalar allows cross-partition
                # bases, tensor_tensor below does not
                den0 = work.tile([1, width], F32, tag="rd", name="den0")
                nc.vector.tensor_scalar(out=den0.bitcast(I32), in0=nxz,
                                        scalar1=m1c, scalar2=None,
                                        op0=ALU.bitwise_xor)
                z = work.tile([1, width], F32, tag="rz", name="rz")
                nc.vector.tensor_scalar_mul(out=z, in0=nxz.bitcast(F32),
                                            scalar1=0.23549792)
                u = work.tile([1, width], F32, tag="ru", name="ru")
                nc.vector.tensor_mul(u, den0, z)
                nc.vector.scalar_tensor_tensor(out=z, in0=u, scalar=2.0017324,
                                               in1=z, op0=ALU.add, op1=ALU.mult)
                nc.vector.tensor_mul(u, den0, z)
                nc.vector.scalar_tensor_tensor(out=z, in0=u, scalar=2.0,
                                               in1=z, op0=ALU.add, op1=ALU.mult)
                return z

            def emit_normalize(key, acp_t):
                hh, ts_idx = key
                hsn = slice(CH * hh, CH * (hh + 1))
                den = acp_t[64:65, :, :].rearrange("p a b -> p (a b)")
                z = recip_neg(den, 1024)
                for tg in range(2):
                    tsl = slice(tg * 512, (tg + 1) * 512)
                    rbc = work.tile([64, 512], F32, tag="rbc", name="rbc")
                    nc.gpsimd.partition_broadcast(rbc, z[:, tg * 512:(tg + 1) * 512])
                    nc.vector.scalar_tensor_tensor(
                        out=a_cats[ts_idx][hsn, tsl], in0=acp_t[0:64, tg, :],
                        scalar=-1.0, in1=rbc, op0=ALU.mult, op1=ALU.mult)

            def emit_proj_unit(ts_idx, u, tag="pp", on_act=False):
                # one [128,512] unit of the projection of t-stripe ts_idx.
                # on_act: do the PSUM->SBUF cast on the scalar engine (only
                # sensible in the tail, after the last exp, when ACT is idle)
                m, n = u >> 1, u & 1
                tb = ts_idx * TSUP
                ms = slice(m * 128, (m + 1) * 128)
                pp = ps.tile([128, 512], F32, tag=tag, bufs=2, name="pp")
                nc.tensor.matmul(pp, sb_pw[:, ms],
                                 a_cats[ts_idx][:, n * 512:(n + 1) * 512],
                                 start=True, stop=True)
                pt = work.tile([128, 512], BF16, tag="pt", bufs=4, name="pt")
                if on_act:
                    nc.scalar.activation(out=pt, in_=pp, func=AF.Copy)
                else:
                    nc.vector.tensor_scalar_add(out=pt, in0=pp, scalar1=0.0)
                nc.sync.dma_start(out=part[ms, tb + n * 512:tb + (n + 1) * 512], in_=pt)

            pending_norm = None   # (key, acp) not yet normalized
            for tsup in range(NT):
                t0 = tsup * TSUP
                for h in range(HEADS_PER_CORE):
                    hs = slice(CH * h, CH * (h + 1))
                    vs = slice(80 * h, 80 * h + 65)
                    apl = []
                    for tg in range(2):
                        ap_t = ps.tile([65, 512], F32, tag=f"apl{tg}", name=f"apl{tg}")
                        apl.append(ap_t)
                    prevE = None
                    for j in range(SJ + 1):
                        if j == 1 and pending_norm is not None:
                            emit_normalize(*pending_norm)
                            pending_norm = None
                        if h == 1 and tsup > 0 and 14 <= j < 30 and (j - 14) % 2 == 0:
                            emit_proj_unit(tsup - 1, (j - 14) // 2)
                        if tsup == 0 and h == 0:
                            if j in (3, 7) and j < SJ:
                                emit_q(2 + (j - 3) // 4)
                            if j in (2, 6, 10, 14, 18, 22) and j < SJ:
                                emit_v(2 + (j - 2) // 4)
                        if tsup == 0 and h == 1:
                            if j in (3, 7, 11, 15) and j < SJ:
                                emit_q(4 + (j - 3) // 4)
                        if j < SJ:
                            js = slice(j * 128, (j + 1) * 128)
                            sc = ps.tile([128, 1024], F32, tag="sc", bufs=2, name="sc")
                            nc.tensor.matmul(sc[:, 0:512], k2[hs, js],
                                             q2[hs, t0:t0 + 512], start=True, stop=True)
                            nc.tensor.matmul(sc[:, 512:1024], k2[hs, js],
                                             q2[hs, t0 + 512:t0 + 1024],
                                             start=True, stop=True)
                            if j % 6 == 5:
                                # Schraudolph exp on the DVE: bf16(i16(s*a+b))
                                # ~= exp(0.125*s); relieves the ACT bottleneck
                                # (whole-pipeline rel err 3.8e-4 vs 3.5e-4)
                                Ei = work.tile([128, 1024], I16, tag="Ei",
                                               bufs=3, name="Ei")
                                nc.vector.tensor_scalar(
                                    out=Ei, in0=sc, scalar1=23.08306164,
                                    scalar2=16251.0, op0=ALU.mult, op1=ALU.add)
                                E = Ei.bitcast(BF16)
                            else:
                                E = work.tile([128, 1024], BF16, tag="E",
                                              bufs=8, name="E")
                                nc.scalar.activation(out=E, in_=sc, func=AF.Exp,
                                                     scale=0.125)
                        if prevE is not None:
                            jj = j - 1
                            st = (jj == 0)
                            sp = (jj == SJ - 1)
                            nc.tensor.matmul(apl[0], vt[:, jj, vs],
                                             prevE[:, 0:512], start=st, stop=sp)
                            nc.tensor.matmul(apl[1], vt[:, jj, vs],
                                             prevE[:, 512:1024], start=st, stop=sp)
                        prevE = E if j < SJ else None
                    if tsup == NT - 1 and h == 1:
                        # final stripe: normalize straight out of PSUM and
                        # interleave the last projection per t-half
                        for tg in range(2):
                            tsl = slice(tg * 512, (tg + 1) * 512)
                            # recip on the now-idle ACT engine: 1/d = exp(-ln d)
                            lnd = work.tile([1, 512], F32, tag="lnd", name="lnd")
                            nc.scalar.activation(out=lnd, in_=apl[tg][64:65, :],
                                                 func=AF.Ln)
                            rec = work.tile([1, 512], F32, tag="rec", name="rec")
                            nc.scalar.activation(out=rec, in_=lnd, func=AF.Exp,
                                                 scale=-1.0)
                            rbc = work.tile([64, 512], F32, tag="rbc", name="rbc")
                            nc.gpsimd.partition_broadcast(rbc, rec)
                            nc.vector.tensor_mul(a_cats[NT - 1][hs, tsl],
                                                 apl[tg][0:64, :], rbc)
                            for m in range(4):
                                emit_proj_unit(NT - 1, 2 * m + tg,
                                               tag=("pp", "sc")[m % 2],
                                               on_act=True)
                    else:
                        # move a_plus off PSUM quickly (releases the apl banks)
                        acp = work.tile([65, 2, 512], F32, tag="acp", bufs=3, name="acp")
                        for tg in range(2):
                            nc.vector.tensor_copy(out=acp[:, tg, :], in_=apl[tg])
                        pending_norm = ((h, tsup), acp)

    nc.compile()
    return nc


def get_program():
    global _PROGRAM
    if _PROGRAM is None:
        _PROGRAM = build_program()
    return _PROGRAM


def make_in_maps(x, norm_w, norm_b, qkv_w, qkv_b, proj_w):
    """Build the 8 per-core input maps from full inputs."""
    f = np.float32
    bf = ml_dtypes.bfloat16
    # [B, 128, 4, L]: partition-major so each partition's x DMA is one
    # contiguous 32KB run (channel = t*128 + p)
    x2 = np.ascontiguousarray(
        x.reshape(B, 4, 128, L).transpose(0, 2, 1, 3), dtype=bf)

    gmask = np.zeros((128, 4, G), dtype=f)
    bmask = np.zeros((G, 4, 128), dtype=f)
    for t in range(4):
        for p in range(128):
            g = (t * 128 + p) // 16
            gmask[p, t, g] = 1.0 / 16.0
            bmask[g, t, p] = 1.0
    gamma4 = np.ascontiguousarray(norm_w.reshape(4, 128), dtype=f)
    beta4 = np.ascontiguousarray(norm_b.reshape(4, 128), dtype=f)

    in_maps = []
    for cid in range(N_CORES):
        b = cid // 4
        h0 = 2 * (cid % 4)
        h1 = h0 + 1
        qrows = list(range(192 * h0, 192 * h0 + 64)) + list(range(192 * h1, 192 * h1 + 64))
        krows = [r + 64 for r in qrows]
        vrows = [r + 128 for r in qrows]
        wqT = np.ascontiguousarray(qkv_w[qrows, :].T, dtype=bf)
        wkT = np.ascontiguousarray(qkv_w[krows, :].T, dtype=bf)
        wvT = np.ascontiguousarray(qkv_w[vrows, :].T, dtype=bf)
        qbv = np.ascontiguousarray(qkv_b[qrows], dtype=f)
        kbv = np.ascontiguousarray(qkv_b[krows], dtype=f)
        vbv = np.ascontiguousarray(qkv_b[vrows], dtype=f)
        ch_cols = list(range(64 * h0, 64 * h0 + 64)) + list(range(64 * h1, 64 * h1 + 64))
        pwT = np.ascontiguousarray(proj_w[:, ch_cols].T, dtype=bf)
        in_maps.append({
            "xb": x2[b], "gmask": gmask, "bmask": bmask,
            "gamma4": gamma4, "beta4": beta4,
            "wqT": wqT, "wkT": wkT, "wvT": wvT,
            "qb": qbv, "kb": kbv, "vb": vbv, "pwT": pwT,
        })
    return in_maps


def kernel(x, norm_w, norm_b, qkv_w, qkv_b, proj_w, proj_b, _trace=False):
    x = np.asarray(x, dtype=np.float32)
    in_maps = make_in_maps(x, np.asarray(norm_w), np.asarray(norm_b),
                           np.asarray(qkv_w), np.asarray(qkv_b), np.asarray(proj_w))
    nc = get_program()
    res = run_bass_kernel_spmd(nc, in_maps, list(range(N_CORES)), trace=_trace)
    hout = np.zeros((B, C, L), dtype=np.float32)
    for cid in range(N_CORES):
        hout[cid // 4] += np.asarray(res.results[cid]["part"], dtype=np.float32)
    hout += np.asarray(proj_b, dtype=np.float32)[None, :, None]
    out = x + hout.reshape(x.shape)
    if _trace:
        return out.astype(np.float32), res
    return out.astype(np.float32)


# revision 17
# speedup vs baseline: 1.4434x; 1.4434x over previous
"""AttentionBlock (GroupNorm + QKV + 8-head spatial attention + proj + residual)
on 8 Trainium2 NeuronCores.

Sharding: 16 head-batches (B=2 x NH=8) are split 2-per-core; cores 0-3 take
batch 0, cores 4-7 batch 1.  Each core:
  - loads its batch's x [512, 4096] (bf16), GroupNorm statistics on-chip via
    bn_stats tracking the DMA chunk by chunk; warm-up matmuls keep the PE
    p-state high through the load,
  - folds the GroupNorm affine into the QKV weights (W' = W*A per channel,
    bias' = W@B + qkv_b) so x feeds the QKV matmuls directly,
  - computes q/k/v in [c, L] layout; v is DMA-transposed (hardware XBAR)
    into vT [s, c] chunks so the PE never burns cycles transposing; the
    softmax-denominator ones-columns are memset once,
  - scores are computed in [s, t] layout with K=64 partition-sliced matmuls;
    exp (ACT) is the bottleneck and runs back-to-back; the a_plus
    accumulation lags one j so the in-order PE stream never stalls,
  - the projection of each finished t-stripe goes to its own a_cat tile
    (no false deps) and is spread one [128,512] unit at a time over the
    next h==1 stripe's j-loop; partials stream out over DMA in bf16.
Host sums the 4 partials per batch, adds proj_b and the residual.

All big matmuls run in bf16 (PSUM accumulation stays fp32).  Measured
accuracy ~3.5e-4 rel vs the 2e-2 gate.  Steady state is ACT(exp)-bound:
256 exps of [128,1024] back-to-back at ~1.1us each.
"""

import numpy as np
import ml_dtypes

import concourse.bacc as bacc
import concourse.tile as tile
from concourse import mybir
from concourse.bass_utils import run_bass_kernel_spmd

B, C = 2, 512
L = 64 * 64           # 4096
NH = 8                # heads total
CH = 64               # channels per head
G = 32                # groups
EPS = 1e-5
N_CORES = 8
HEADS_PER_CORE = 2

F32 = mybir.dt.float32
F32R = mybir.dt.float32r
BF16 = mybir.dt.bfloat16
I32 = mybir.dt.int32
I16 = mybir.dt.int16
AF = mybir.ActivationFunctionType
ALU = mybir.AluOpType

TSUP = 1024           # t-stripe width (2 PSUM banks)
NT = L // TSUP        # 4 stripes
SJ = 32               # number of 128-wide s-chunks

_PROGRAM = None


def build_program():
    nc = bacc.Bacc()
    xb = nc.declare_dram_parameter("xb", [128, 4, L], BF16, isOutput=False).ap()
    gmask = nc.declare_dram_parameter("gmask", [128, 4, G], F32R, isOutput=False).ap()
    bmask = nc.declare_dram_parameter("bmask", [G, 4, 128], F32R, isOutput=False).ap()
    gamma4 = nc.declare_dram_parameter("gamma4", [4, 128], F32, isOutput=False).ap()
    beta4 = nc.declare_dram_parameter("beta4", [4, 128], F32, isOutput=False).ap()
    wqT = nc.declare_dram_parameter("wqT", [C, 128], BF16, isOutput=False).ap()
    wkT = nc.declare_dram_parameter("wkT", [C, 128], BF16, isOutput=False).ap()
    wvT = nc.declare_dram_parameter("wvT", [C, 128], BF16, isOutput=False).ap()
    qb = nc.declare_dram_parameter("qb", [128], F32, isOutput=False).ap()
    kb = nc.declare_dram_parameter("kb", [128], F32, isOutput=False).ap()
    vb = nc.declare_dram_parameter("vb", [128], F32, isOutput=False).ap()
    pwT = nc.declare_dram_parameter("pwT", [128, C], BF16, isOutput=False).ap()
    part = nc.declare_dram_parameter("part", [C, L], BF16, isOutput=True).ap()

    with tile.TileContext(nc) as tc:
        with (
            tc.tile_pool(name="consts", bufs=1) as consts,
            tc.tile_pool(name="big", bufs=1) as big,
            tc.tile_pool(name="work", bufs=2) as work,
            tc.tile_pool(name="ps", bufs=1, space="PSUM") as ps,
        ):
            # ---- small consts needed for GroupNorm stats first ----
            sb_gmask = consts.tile([128, 4, G], F32R)
            nc.sync.dma_start(out=sb_gmask, in_=gmask)
            sb_bmask = consts.tile([G, 4, 128], F32R)
            nc.sync.dma_start(out=sb_bmask, in_=bmask)
            sb_gamma = consts.tile([128, 4], F32)
            nc.sync.dma_start(out=sb_gamma, in_=gamma4.rearrange("t p -> p t"))
            sb_beta = consts.tile([128, 4], F32)
            nc.sync.dma_start(out=sb_beta, in_=beta4.rearrange("t p -> p t"))
            eps32 = consts.tile([32, 1], F32)
            nc.vector.memset(eps32, EPS)
            m1c = consts.tile([1, 1], I32)
            nc.vector.memset(m1c, -1)
            dummy_w = consts.tile([128, 128], BF16)
            nc.vector.memset(dummy_w, 0.0)

            # ---- load x; bn_stats tracks the DMA; PE warm-up in parallel ----
            xt = big.tile([128, 4, L], BF16)
            stats = work.tile([128, 4, 8, 6], F32, bufs=1)
            for s in range(8):
                ns = slice(s * 512, (s + 1) * 512)
                nc.sync.dma_start(out=xt[:, :, ns], in_=xb[:, :, ns])
                for t in range(4):
                    nc.vector.bn_stats(out=stats[:, t, s, :], in_=xt[:, t, ns])
                if s >= 1:
                    # p-state warm-up: harmless matmuls paced by the DMA so the
                    # PE clock is at full speed when the QKV burst starts
                    for _ in range(6):
                        wps = ps.tile([128, 512], F32, tag="pp", bufs=2, name="wps")
                        nc.tensor.matmul(wps, dummy_w, xt[:, 0, ns],
                                         start=True, stop=True)
            for _ in range(10):
                wps = ps.tile([128, 512], F32, tag="pp", bufs=2, name="wps")
                nc.tensor.matmul(wps, dummy_w, xt[:, 0, 3584:4096],
                                 start=True, stop=True)
            mv = work.tile([128, 4, 2], F32, bufs=1)
            for t in range(4):
                nc.vector.bn_aggr(out=mv[:, t, :], in_=stats[:, t, :, :])
            # per-channel [mean, var+mean^2]
            stats2 = work.tile([128, 4, 2], F32R, bufs=1)
            msq = work.tile([128, 4, 1], F32, bufs=1)
            nc.vector.tensor_copy(out=stats2[:, :, 0:1], in_=mv[:, :, 0:1])
            nc.vector.tensor_mul(msq, mv[:, :, 0:1], mv[:, :, 0:1])
            nc.vector.tensor_add(stats2[:, :, 1:2], mv[:, :, 1:2], msq)
            # group stats via mask matmul: [32, 2] = (mean_g, E[x^2]_g)
            gps = ps.tile([32, 2], F32, tag="apl0")
            for t in range(4):
                nc.tensor.matmul(
                    gps, sb_gmask[:, t, :], stats2[:, t, :],
                    start=(t == 0), stop=(t == 3),
                )
            gs = work.tile([32, 2], F32, bufs=1)
            nc.vector.tensor_copy(out=gs, in_=gps)
            msqg = work.tile([32, 1], F32, bufs=1)
            varg = work.tile([32, 1], F32, bufs=1)
            nc.vector.tensor_mul(msqg, gs[:, 0:1], gs[:, 0:1])
            nc.vector.tensor_sub(varg, gs[:, 1:2], msqg)
            # rstd = exp(-0.5*ln(var+eps))  (Ln+Exp share one ACT table set)
            lng = work.tile([32, 1], F32, bufs=1)
            nc.scalar.activation(out=lng, in_=varg, func=AF.Ln, bias=eps32, scale=1.0)
            rstdg = work.tile([32, 1], F32, bufs=1)
            nc.scalar.activation(out=rstdg, in_=lng, func=AF.Exp, scale=-0.5)
            gstats2 = work.tile([32, 2], F32R, bufs=1)
            nc.vector.tensor_copy(out=gstats2[:, 0:1], in_=gs[:, 0:1])
            nc.vector.tensor_copy(out=gstats2[:, 1:2], in_=rstdg)

            # ---- weights (arrive during/after x) ----
            sb_wq = consts.tile([128, 4, 128], BF16)
            nc.sync.dma_start(out=sb_wq, in_=wqT.rearrange("(kk p) m -> p kk m", p=128))
            sb_wk = consts.tile([128, 4, 128], BF16)
            nc.sync.dma_start(out=sb_wk, in_=wkT.rearrange("(kk p) m -> p kk m", p=128))
            sb_wv = consts.tile([128, 4, 128], BF16)
            nc.sync.dma_start(out=sb_wv, in_=wvT.rearrange("(kk p) m -> p kk m", p=128))
            sb_pw = consts.tile([128, C], BF16)
            nc.sync.dma_start(out=sb_pw, in_=pwT)
            sb_qb = consts.tile([128, 1], F32)
            nc.sync.dma_start(out=sb_qb, in_=qb.unsqueeze(1))
            sb_kb = consts.tile([128, 1], F32)
            nc.sync.dma_start(out=sb_kb, in_=kb.unsqueeze(1))
            sb_vb = consts.tile([128, 1], F32)
            nc.sync.dma_start(out=sb_vb, in_=vb.unsqueeze(1))

            # ---- per-channel affine A, Bs  (hid = x*A + Bs) ----
            A_all = work.tile([128, 4], F32, bufs=1)
            Bcol = work.tile([128, 4, 2], BF16, bufs=1)
            for t in range(4):
                cst = ps.tile([128, 2], F32, tag="apl1")
                nc.tensor.matmul(
                    cst, sb_bmask[:, t, :], gstats2, start=True, stop=True
                )
                nc.vector.tensor_mul(A_all[:, t:t + 1], cst[:, 1:2], sb_gamma[:, t:t + 1])
                tmp = work.tile([128, 1], F32, tag="tmp")
                nc.vector.tensor_mul(tmp, cst[:, 0:1], A_all[:, t:t + 1])
                nc.vector.tensor_sub(Bcol[:, t, :], sb_beta[:, t:t + 1].broadcast_to([128, 2]), tmp.broadcast_to([128, 2]))

            # ---- fold affine into QKV weights ----
            # bias' = W^T @ Bs + b first (reads original W), then W *= A in place
            cq_ps = ps.tile([128, 2], F32, tag="sc", bufs=2)
            ck_ps = ps.tile([128, 2], F32, tag="apl0")
            cv_ps = ps.tile([128, 2], F32, tag="apl1")
            for t in range(4):
                nc.tensor.matmul(cq_ps, sb_wq[:, t, :], Bcol[:, t, :],
                                 start=(t == 0), stop=(t == 3))
                nc.tensor.matmul(ck_ps, sb_wk[:, t, :], Bcol[:, t, :],
                                 start=(t == 0), stop=(t == 3))
                nc.tensor.matmul(cv_ps, sb_wv[:, t, :], Bcol[:, t, :],
                                 start=(t == 0), stop=(t == 3))
            qc = consts.tile([128, 1], F32)
            nc.vector.tensor_add(qc, cq_ps[:, 0:1], sb_qb)
            kc = consts.tile([128, 1], F32)
            nc.vector.tensor_add(kc, ck_ps[:, 0:1], sb_kb)
            vc_b = consts.tile([128, 1], F32)
            nc.vector.tensor_add(vc_b, cv_ps[:, 0:1], sb_vb)
            for t in range(4):
                nc.vector.tensor_scalar_mul(
                    out=sb_wq[:, t, :], in0=sb_wq[:, t, :], scalar1=A_all[:, t:t + 1])
                nc.vector.tensor_scalar_mul(
                    out=sb_wk[:, t, :], in0=sb_wk[:, t, :], scalar1=A_all[:, t:t + 1])
                nc.vector.tensor_scalar_mul(
                    out=sb_wv[:, t, :], in0=sb_wv[:, t, :], scalar1=A_all[:, t:t + 1])

            for _ in range(4):
                wps = ps.tile([128, 512], F32, tag="pp", bufs=2, name="wps")
                nc.tensor.matmul(wps, dummy_w, xt[:, 0, 3584:4096],
                                 start=True, stop=True)
            # ---- QKV in [c, L] layout; vT via hardware XBAR transpose ----
            q2 = big.tile([128, L], BF16)
            k2 = big.tile([128, L], BF16)
            vc2 = big.tile([128, L], BF16)
            # vT: [s, c] both heads + ones cols at 64 (h0) / 129 (h1)
            vt = big.tile([128, SJ, 160], BF16)
            nc.vector.memset(vt[:, :, 64:65], 1.0)
            nc.vector.memset(vt[:, :, 144:145], 1.0)

            def emit_q(n):
                ns = slice(n * 512, (n + 1) * 512)
                qp = ps.tile([128, 512], F32, tag="pp", bufs=2, name="qp")
                for kk in range(4):
                    nc.tensor.matmul(qp, sb_wq[:, kk, :], xt[:, kk, ns],
                                     start=(kk == 0), stop=(kk == 3))
                nc.vector.tensor_scalar_add(out=q2[:, ns], in0=qp, scalar1=qc)

            def emit_v(n):
                ns = slice(n * 512, (n + 1) * 512)
                vp = ps.tile([128, 512], F32, tag="pp", bufs=2, name="vp")
                for kk in range(4):
                    nc.tensor.matmul(vp, sb_wv[:, kk, :], xt[:, kk, ns],
                                     start=(kk == 0), stop=(kk == 3))
                nc.vector.tensor_scalar_add(out=vc2[:, ns], in0=vp, scalar1=vc_b)
                # XBAR-transpose this 512-col span into 4 vt chunks per head
                cs = slice(4 * n, 4 * n + 4)
                nc.sync.dma_start_transpose(out=vt[:, cs, 0:64], in_=vc2[0:64, ns])
                nc.sync.dma_start_transpose(out=vt[:, cs, 80:144], in_=vc2[64:128, ns])

            for n in range(8):
                ns = slice(n * 512, (n + 1) * 512)
                kp = ps.tile([128, 512], F32, tag="pp", bufs=2, name="kp")
                for kk in range(4):
                    nc.tensor.matmul(kp, sb_wk[:, kk, :], xt[:, kk, ns],
                                     start=(kk == 0), stop=(kk == 3))
                nc.vector.tensor_scalar_add(out=k2[:, ns], in0=kp, scalar1=kc)
            emit_q(0)
            emit_q(1)
            emit_v(0)
            emit_v(1)

            # ---- attention ----
            # Per (h, tsup) stripe of 1024 t-columns.  exp (ACT) is the
            # bottleneck and runs back-to-back; the a_plus accumulation lags
            # one j.  Remaining q/v chunks stream into stripe (0,0)'s slack.
            a_cats = [big.tile([128, TSUP], BF16, name=f"a_cat{i}")
                      for i in range(NT)]

            def recip_neg(den, width):
                """z = -1/den at ~18 bits via NOT-seed + 2 NR steps, all as
                plain DVE ops (the scheduler models these accurately, unlike
                the 8-pass InstReciprocal)."""
                nxz = work.tile([1, width], I32, tag="nx", name="nxz")
                nc.vector.tensor_scalar(out=nxz, in0=den.bitcast(I32),
                                        scalar1=m1c, scalar2=None,
                                        op0=ALU.bitwise_xor)
                # partition-0 copy of den (den may live on partition 64):
                # den0 = NOT(NOT(den)) — tensor_scalar allows cross-partition
                # bases, tensor_tensor below does not
                den0 = work.tile([1, width], F32, tag="rd", name="den0")
                nc.vector.tensor_scalar(out=den0.bitcast(I32), in0=nxz,
                                        scalar1=m1c, scalar2=None,
                                        op0=ALU.bitwise_xor)
                z = work.tile([1, width], F32, tag="rz", name="rz")
                nc.vector.tensor_scalar_mul(out=z, in0=nxz.bitcast(F32),
                                            scalar1=0.23549792)
                u = work.tile([1, width], F32, tag="ru", name="ru")
                nc.vector.tensor_mul(u, den0, z)
                nc.vector.scalar_tensor_tensor(out=z, in0=u, scalar=2.0017324,
                                               in1=z, op0=ALU.add, op1=ALU.mult)
                nc.vector.tensor_mul(u, den0, z)
                nc.vector.scalar_tensor_tensor(out=z, in0=u, scalar=2.0,
                                               in1=z, op0=ALU.add, op1=ALU.mult)
                return z

            def emit_normalize(key, acp_t):
                hh, ts_idx = key
                hsn = slice(CH * hh, CH * (hh + 1))
                den = acp_t[64:65, :, :].rearrange("p a b -> p (a b)")
                z = recip_neg(den, 1024)
                for tg in range(2):
                    tsl = slice(tg * 512, (tg + 1) * 512)
                    rbc = work.tile([64, 512], F32, tag="rbc", name="rbc")
                    nc.gpsimd.partition_broadcast(rbc, z[:, tg * 512:(tg + 1) * 512])
                    nc.vector.scalar_tensor_tensor(
                        out=a_cats[ts_idx][hsn, tsl], in0=acp_t[0:64, tg, :],
                        scalar=-1.0, in1=rbc, op0=ALU.mult, op1=ALU.mult)

            def emit_proj_unit(ts_idx, u, tag="pp", on_act=False):
                # one [128,512] unit of the projection of t-stripe ts_idx.
                # on_act: do the PSUM->SBUF cast on the scalar engine (only
                # sensible in the tail, after the last exp, when ACT is idle)
                m, n = u >> 1, u & 1
                tb = ts_idx * TSUP
                ms = slice(m * 128, (m + 1) * 128)
                pp = ps.tile([128, 512], F32, tag=tag, bufs=2, name="pp")
                nc.tensor.matmul(pp, sb_pw[:, ms],
                                 a_cats[ts_idx][:, n * 512:(n + 1) * 512],
                                 start=True, stop=True)
                pt = work.tile([128, 512], BF16, tag="pt", bufs=4, name="pt")
                if on_act:
                    nc.scalar.activation(out=pt, in_=pp, func=AF.Copy)
                else:
                    nc.vector.tensor_scalar_add(out=pt, in0=pp, scalar1=0.0)
                nc.sync.dma_start(out=part[ms, tb + n * 512:tb + (n + 1) * 512], in_=pt)

            pending_norm = None   # (key, acp) not yet normalized
            for tsup in range(NT):
                t0 = tsup * TSUP
                for h in range(HEADS_PER_CORE):
                    hs = slice(CH * h, CH * (h + 1))
                    vs = slice(80 * h, 80 * h + 65)
                    apl = []
                    for tg in range(2):
                        ap_t = ps.tile([65, 512], F32, tag=f"apl{tg}", name=f"apl{tg}")
                        apl.append(ap_t)
                    # av consumption schedule: ACT-exp tiles at lag 1, DVE
                    # (Schraudolph) tiles at lag 3 so the cross-engine latency
                    # of the DVE path never stalls the in-order PE stream
                    av_sched = {}
                    for jj in range(SJ):
                        lag = 3 if jj % 6 == 5 else 1
                        av_sched.setdefault(jj + lag, []).append(jj)
                    av_flat = [jj for i in sorted(av_sched) for jj in av_sched[i]]
                    e_tiles = {}
                    for j in range(SJ + 3):
                        if j == 1 and pending_norm is not None:
                            emit_normalize(*pending_norm)
                            pending_norm = None
                        if h == 1 and tsup > 0 and 14 <= j < 30 and (j - 14) % 2 == 0:
                            emit_proj_unit(tsup - 1, (j - 14) // 2)
                        if tsup == 0 and h == 0:
                            if j in (3, 7) and j < SJ:
                                emit_q(2 + (j - 3) // 4)
                            if j in (2, 6, 10, 14, 18, 22) and j < SJ:
                                emit_v(2 + (j - 2) // 4)
                        if tsup == 0 and h == 1:
                            if j in (3, 7, 11, 15) and j < SJ:
                                emit_q(4 + (j - 3) // 4)
                        if j < SJ:
                            js = slice(j * 128, (j + 1) * 128)
                            sc = ps.tile([128, 1024], F32, tag="sc", bufs=2, name="sc")
                            nc.tensor.matmul(sc[:, 0:512], k2[hs, js],
                                             q2[hs, t0:t0 + 512], start=True, stop=True)
                            nc.tensor.matmul(sc[:, 512:1024], k2[hs, js],
                                             q2[hs, t0 + 512:t0 + 1024],
                                             start=True, stop=True)
                            if j % 6 == 5:
                                # Schraudolph exp on the DVE: bf16(i16(s*a+b))
                                # ~= exp(0.125*s); relieves the ACT bottleneck
                                # (whole-pipeline rel err 3.8e-4 vs 3.5e-4)
                                Ei = work.tile([128, 1024], I16, tag="Ei",
                                               bufs=3, name="Ei")
                                nc.vector.tensor_scalar(
                                    out=Ei, in0=sc, scalar1=23.08306164,
                                    scalar2=16251.0, op0=ALU.mult, op1=ALU.add)
                                E = Ei.bitcast(BF16)
                            else:
                                E = work.tile([128, 1024], BF16, tag="E",
                                              bufs=8, name="E")
                                nc.scalar.activation(out=E, in_=sc, func=AF.Exp,
                                                     scale=0.125)
                            e_tiles[j] = E
                        for jj in av_sched.get(j, []):
                            st = (jj == av_flat[0])
                            sp = (jj == av_flat[-1])
                            Ej = e_tiles.pop(jj)
                            nc.tensor.matmul(apl[0], vt[:, jj, vs],
                                             Ej[:, 0:512], start=st, stop=sp)
                            nc.tensor.matmul(apl[1], vt[:, jj, vs],
                                             Ej[:, 512:1024], start=st, stop=sp)
                    if tsup == NT - 1 and h == 1:
                        # final stripe: normalize straight out of PSUM and
                        # interleave the last projection per t-half
                        for tg in range(2):
                            tsl = slice(tg * 512, (tg + 1) * 512)
                            # recip on the now-idle ACT engine: 1/d = exp(-ln d)
                            lnd = work.tile([1, 512], F32, tag="lnd", name="lnd")
                            nc.scalar.activation(out=lnd, in_=apl[tg][64:65, :],
                                                 func=AF.Ln)
                            rec = work.tile([1, 512], F32, tag="rec", name="rec")
                            nc.scalar.activation(out=rec, in_=lnd, func=AF.Exp,
                                                 scale=-1.0)
                            rbc = work.tile([64, 512], F32, tag="rbc", name="rbc")
                            nc.gpsimd.partition_broadcast(rbc, rec)
                            nc.vector.tensor_mul(a_cats[NT - 1][hs, tsl],
                                                 apl[tg][0:64, :], rbc)
                            for m in range(4):
                                emit_proj_unit(NT - 1, 2 * m + tg,
                                               tag=("pp", "sc")[m % 2],
                                               on_act=True)
                    else:
                        # move a_plus off PSUM quickly (releases the apl banks)
                        acp = work.tile([65, 2, 512], F32, tag="acp", bufs=3, name="acp")
                        for tg in range(2):
                            nc.vector.tensor_copy(out=acp[:, tg, :], in_=apl[tg])
                        pending_norm = ((h, tsup), acp)

    nc.compile()
    return nc


def get_program():
    global _PROGRAM
    if _PROGRAM is None:
        _PROGRAM = build_program()
    return _PROGRAM


def make_in_maps(x, norm_w, norm_b, qkv_w, qkv_b, proj_w):
    """Build the 8 per-core input maps from full inputs."""
    f = np.float32
    bf = ml_dtypes.bfloat16
    # [B, 128, 4, L]: partition-major so each partition's x DMA is one
    # contiguous 32KB run (channel = t*128 + p)
    x2 = np.ascontiguousarray(
        x.reshape(B, 4, 128, L).transpose(0, 2, 1, 3), dtype=bf)

    gmask = np.zeros((128, 4, G), dtype=f)
    bmask = np.zeros((G, 4, 128), dtype=f)
    for t in range(4):
        for p in range(128):
            g = (t * 128 + p) // 16
            gmask[p, t, g] = 1.0 / 16.0
            bmask[g, t, p] = 1.0
    gamma4 = np.ascontiguousarray(norm_w.reshape(4, 128), dtype=f)
    beta4 = np.ascontiguousarray(norm_b.reshape(4, 128), dtype=f)

    in_maps = []
    for cid in range(N_CORES):
        b = cid // 4
        h0 = 2 * (cid % 4)
        h1 = h0 + 1
        qrows = list(range(192 * h0, 192 * h0 + 64)) + list(range(192 * h1, 192 * h1 + 64))
        krows = [r + 64 for r in qrows]
        vrows = [r + 128 for r in qrows]
        wqT = np.ascontiguousarray(qkv_w[qrows, :].T, dtype=bf)
        wkT = np.ascontiguousarray(qkv_w[krows, :].T, dtype=bf)
        wvT = np.ascontiguousarray(qkv_w[vrows, :].T, dtype=bf)
        qbv = np.ascontiguousarray(qkv_b[qrows], dtype=f)
        kbv = np.ascontiguousarray(qkv_b[krows], dtype=f)
        vbv = np.ascontiguousarray(qkv_b[vrows], dtype=f)
        ch_cols = list(range(64 * h0, 64 * h0 + 64)) + list(range(64 * h1, 64 * h1 + 64))
        pwT = np.ascontiguousarray(proj_w[:, ch_cols].T, dtype=bf)
        in_maps.append({
            "xb": x2[b], "gmask": gmask, "bmask": bmask,
            "gamma4": gamma4, "beta4": beta4,
            "wqT": wqT, "wkT": wkT, "wvT": wvT,
            "qb": qbv, "kb": kbv, "vb": vbv, "pwT": pwT,
        })
    return in_maps


def kernel(x, norm_w, norm_b, qkv_w, qkv_b, proj_w, proj_b, _trace=False):
    x = np.asarray(x, dtype=np.float32)
    in_maps = make_in_maps(x, np.asarray(norm_w), np.asarray(norm_b),
                           np.asarray(qkv_w), np.asarray(qkv_b), np.asarray(proj_w))
    nc = get_program()
    res = run_bass_kernel_spmd(nc, in_maps, list(range(N_CORES)), trace=_trace)
    hout = np.zeros((B, C, L), dtype=np.float32)
    for cid in range(N_CORES):
        hout[cid // 4] += np.asarray(res.results[cid]["part"], dtype=np.float32)
    hout += np.asarray(proj_b, dtype=np.float32)[None, :, None]
    out = x + hout.reshape(x.shape)
    if _trace:
        return out.astype(np.float32), res
    return out.astype(np.float32)


# revision 18
# speedup vs baseline: 1.5120x; 1.0476x over previous
"""AttentionBlock (GroupNorm + QKV + 8-head spatial attention + proj + residual)
on 8 Trainium2 NeuronCores.

Sharding: 16 head-batches (B=2 x NH=8) are split 2-per-core; cores 0-3 take
batch 0, cores 4-7 batch 1.  Each core:
  - loads its batch's x [512, 4096] (bf16), GroupNorm statistics on-chip via
    bn_stats tracking the DMA chunk by chunk; warm-up matmuls keep the PE
    p-state high through the load,
  - folds the GroupNorm affine into the QKV weights (W' = W*A per channel,
    bias' = W@B + qkv_b) so x feeds the QKV matmuls directly,
  - computes q/k/v in [c, L] layout; v is DMA-transposed (hardware XBAR)
    into vT [s, c] chunks so the PE never burns cycles transposing; the
    softmax-denominator ones-columns are memset once,
  - scores are computed in [s, t] layout with K=64 partition-sliced matmuls;
    exp (ACT) is the bottleneck and runs back-to-back; the a_plus
    accumulation lags one j so the in-order PE stream never stalls,
  - the projection of each finished t-stripe goes to its own a_cat tile
    (no false deps) and is spread one [128,512] unit at a time over the
    next h==1 stripe's j-loop; partials stream out over DMA in bf16.
Host sums the 4 partials per batch, adds proj_b and the residual.

All big matmuls run in bf16 (PSUM accumulation stays fp32).  Measured
accuracy ~3.5e-4 rel vs the 2e-2 gate.  Steady state is ACT(exp)-bound:
256 exps of [128,1024] back-to-back at ~1.1us each.
"""

import numpy as np
import ml_dtypes

import concourse.bacc as bacc
import concourse.tile as tile
from concourse import mybir
from concourse.bass_utils import run_bass_kernel_spmd

B, C = 2, 512
L = 64 * 64           # 4096
NH = 8                # heads total
CH = 64               # channels per head
G = 32                # groups
EPS = 1e-5
N_CORES = 8
HEADS_PER_CORE = 2

F32 = mybir.dt.float32
F32R = mybir.dt.float32r
BF16 = mybir.dt.bfloat16
I32 = mybir.dt.int32
I16 = mybir.dt.int16
AF = mybir.ActivationFunctionType
ALU = mybir.AluOpType

TSUP = 1024           # t-stripe width (2 PSUM banks)
NT = L // TSUP        # 4 stripes
SJ = 32               # number of 128-wide s-chunks

_PROGRAM = None


def build_program():
    nc = bacc.Bacc()
    xb = nc.declare_dram_parameter("xb", [128, 4, L], BF16, isOutput=False).ap()
    gmask = nc.declare_dram_parameter("gmask", [128, 4, G], F32R, isOutput=False).ap()
    bmask = nc.declare_dram_parameter("bmask", [G, 4, 128], F32R, isOutput=False).ap()
    gamma4 = nc.declare_dram_parameter("gamma4", [4, 128], F32, isOutput=False).ap()
    beta4 = nc.declare_dram_parameter("beta4", [4, 128], F32, isOutput=False).ap()
    wqT = nc.declare_dram_parameter("wqT", [C, 128], BF16, isOutput=False).ap()
    wkT = nc.declare_dram_parameter("wkT", [C, 128], BF16, isOutput=False).ap()
    wvT = nc.declare_dram_parameter("wvT", [C, 128], BF16, isOutput=False).ap()
    qb = nc.declare_dram_parameter("qb", [128], F32, isOutput=False).ap()
    kb = nc.declare_dram_parameter("kb", [128], F32, isOutput=False).ap()
    vb = nc.declare_dram_parameter("vb", [128], F32, isOutput=False).ap()
    pwT = nc.declare_dram_parameter("pwT", [128, C], BF16, isOutput=False).ap()
    part = nc.declare_dram_parameter("part", [C, L], BF16, isOutput=True).ap()

    with tile.TileContext(nc) as tc:
        with (
            tc.tile_pool(name="consts", bufs=1) as consts,
            tc.tile_pool(name="big", bufs=1) as big,
            tc.tile_pool(name="work", bufs=2) as work,
            tc.tile_pool(name="ps", bufs=1, space="PSUM") as ps,
        ):
            # ---- small consts needed for GroupNorm stats first ----
            sb_gmask = consts.tile([128, 4, G], F32R)
            nc.sync.dma_start(out=sb_gmask, in_=gmask)
            sb_bmask = consts.tile([G, 4, 128], F32R)
            nc.sync.dma_start(out=sb_bmask, in_=bmask)
            sb_gamma = consts.tile([128, 4], F32)
            nc.sync.dma_start(out=sb_gamma, in_=gamma4.rearrange("t p -> p t"))
            sb_beta = consts.tile([128, 4], F32)
            nc.sync.dma_start(out=sb_beta, in_=beta4.rearrange("t p -> p t"))
            eps32 = consts.tile([32, 1], F32)
            nc.vector.memset(eps32, EPS)
            m1c = consts.tile([1, 1], I32)
            nc.vector.memset(m1c, -1)
            dummy_w = consts.tile([128, 128], BF16)
            nc.vector.memset(dummy_w, 0.0)

            # ---- load x; bn_stats tracks the DMA; PE warm-up in parallel ----
            xt = big.tile([128, 4, L], BF16)
            stats = work.tile([128, 4, 8, 6], F32, bufs=1)
            for s in range(8):
                ns = slice(s * 512, (s + 1) * 512)
                nc.sync.dma_start(out=xt[:, :, ns], in_=xb[:, :, ns])
                for t in range(4):
                    nc.vector.bn_stats(out=stats[:, t, s, :], in_=xt[:, t, ns])
                if s >= 1:
                    # p-state warm-up: harmless matmuls paced by the DMA so the
                    # PE clock is at full speed when the QKV burst starts
                    for _ in range(6):
                        wps = ps.tile([128, 512], F32, tag="pp", bufs=2, name="wps")
                        nc.tensor.matmul(wps, dummy_w, xt[:, 0, ns],
                                         start=True, stop=True)
            for _ in range(10):
                wps = ps.tile([128, 512], F32, tag="pp", bufs=2, name="wps")
                nc.tensor.matmul(wps, dummy_w, xt[:, 0, 3584:4096],
                                 start=True, stop=True)
            mv = work.tile([128, 4, 2], F32, bufs=1)
            for t in range(4):
                nc.vector.bn_aggr(out=mv[:, t, :], in_=stats[:, t, :, :])
            # per-channel [mean, var+mean^2]
            stats2 = work.tile([128, 4, 2], F32R, bufs=1)
            msq = work.tile([128, 4, 1], F32, bufs=1)
            nc.vector.tensor_copy(out=stats2[:, :, 0:1], in_=mv[:, :, 0:1])
            nc.vector.tensor_mul(msq, mv[:, :, 0:1], mv[:, :, 0:1])
            nc.vector.tensor_add(stats2[:, :, 1:2], mv[:, :, 1:2], msq)
            # group stats via mask matmul: [32, 2] = (mean_g, E[x^2]_g)
            gps = ps.tile([32, 2], F32, tag="apl0")
            for t in range(4):
                nc.tensor.matmul(
                    gps, sb_gmask[:, t, :], stats2[:, t, :],
                    start=(t == 0), stop=(t == 3),
                )
            gs = work.tile([32, 2], F32, bufs=1)
            nc.vector.tensor_copy(out=gs, in_=gps)
            msqg = work.tile([32, 1], F32, bufs=1)
            varg = work.tile([32, 1], F32, bufs=1)
            nc.vector.tensor_mul(msqg, gs[:, 0:1], gs[:, 0:1])
            nc.vector.tensor_sub(varg, gs[:, 1:2], msqg)
            # rstd = exp(-0.5*ln(var+eps))  (Ln+Exp share one ACT table set)
            lng = work.tile([32, 1], F32, bufs=1)
            nc.scalar.activation(out=lng, in_=varg, func=AF.Ln, bias=eps32, scale=1.0)
            rstdg = work.tile([32, 1], F32, bufs=1)
            nc.scalar.activation(out=rstdg, in_=lng, func=AF.Exp, scale=-0.5)
            gstats2 = work.tile([32, 2], F32R, bufs=1)
            nc.vector.tensor_copy(out=gstats2[:, 0:1], in_=gs[:, 0:1])
            nc.vector.tensor_copy(out=gstats2[:, 1:2], in_=rstdg)

            # ---- weights (arrive during/after x) ----
            sb_wq = consts.tile([128, 4, 128], BF16)
            nc.sync.dma_start(out=sb_wq, in_=wqT.rearrange("(kk p) m -> p kk m", p=128))
            sb_wk = consts.tile([128, 4, 128], BF16)
            nc.sync.dma_start(out=sb_wk, in_=wkT.rearrange("(kk p) m -> p kk m", p=128))
            sb_wv = consts.tile([128, 4, 128], BF16)
            nc.sync.dma_start(out=sb_wv, in_=wvT.rearrange("(kk p) m -> p kk m", p=128))
            sb_pw = consts.tile([128, C], BF16)
            nc.sync.dma_start(out=sb_pw, in_=pwT)
            sb_qb = consts.tile([128, 1], F32)
            nc.sync.dma_start(out=sb_qb, in_=qb.unsqueeze(1))
            sb_kb = consts.tile([128, 1], F32)
            nc.sync.dma_start(out=sb_kb, in_=kb.unsqueeze(1))
            sb_vb = consts.tile([128, 1], F32)
            nc.sync.dma_start(out=sb_vb, in_=vb.unsqueeze(1))

            # ---- per-channel affine A, Bs  (hid = x*A + Bs) ----
            A_all = work.tile([128, 4], F32, bufs=1)
            Bcol = work.tile([128, 4, 2], BF16, bufs=1)
            for t in range(4):
                cst = ps.tile([128, 2], F32, tag="apl1")
                nc.tensor.matmul(
                    cst, sb_bmask[:, t, :], gstats2, start=True, stop=True
                )
                nc.vector.tensor_mul(A_all[:, t:t + 1], cst[:, 1:2], sb_gamma[:, t:t + 1])
                tmp = work.tile([128, 1], F32, tag="tmp")
                nc.vector.tensor_mul(tmp, cst[:, 0:1], A_all[:, t:t + 1])
                nc.vector.tensor_sub(Bcol[:, t, :], sb_beta[:, t:t + 1].broadcast_to([128, 2]), tmp.broadcast_to([128, 2]))

            # ---- fold affine into QKV weights ----
            # bias' = W^T @ Bs + b first (reads original W), then W *= A in place
            cq_ps = ps.tile([128, 2], F32, tag="sc", bufs=2)
            ck_ps = ps.tile([128, 2], F32, tag="apl0")
            cv_ps = ps.tile([128, 2], F32, tag="apl1")
            for t in range(4):
                nc.tensor.matmul(cq_ps, sb_wq[:, t, :], Bcol[:, t, :],
                                 start=(t == 0), stop=(t == 3))
                nc.tensor.matmul(ck_ps, sb_wk[:, t, :], Bcol[:, t, :],
                                 start=(t == 0), stop=(t == 3))
                nc.tensor.matmul(cv_ps, sb_wv[:, t, :], Bcol[:, t, :],
                                 start=(t == 0), stop=(t == 3))
            qc = consts.tile([128, 1], F32)
            nc.vector.tensor_add(qc, cq_ps[:, 0:1], sb_qb)
            kc = consts.tile([128, 1], F32)
            nc.vector.tensor_add(kc, ck_ps[:, 0:1], sb_kb)
            vc_b = consts.tile([128, 1], F32)
            nc.vector.tensor_add(vc_b, cv_ps[:, 0:1], sb_vb)
            for t in range(4):
                nc.vector.tensor_scalar_mul(
                    out=sb_wq[:, t, :], in0=sb_wq[:, t, :], scalar1=A_all[:, t:t + 1])
                nc.vector.tensor_scalar_mul(
                    out=sb_wk[:, t, :], in0=sb_wk[:, t, :], scalar1=A_all[:, t:t + 1])
                nc.vector.tensor_scalar_mul(
                    out=sb_wv[:, t, :], in0=sb_wv[:, t, :], scalar1=A_all[:, t:t + 1])

            for _ in range(4):
                wps = ps.tile([128, 512], F32, tag="pp", bufs=2, name="wps")
                nc.tensor.matmul(wps, dummy_w, xt[:, 0, 3584:4096],
                                 start=True, stop=True)
            # ---- QKV in [c, L] layout; vT via hardware XBAR transpose ----
            q2 = big.tile([128, L], BF16)
            k2 = big.tile([128, L], BF16)
            vc2 = big.tile([128, L], BF16)
            # vT: [s, c] both heads + ones cols at 64 (h0) / 129 (h1)
            vt = big.tile([128, SJ, 160], BF16)
            nc.vector.memset(vt[:, :, 64:65], 1.0)
            nc.vector.memset(vt[:, :, 144:145], 1.0)

            def emit_q(n):
                ns = slice(n * 512, (n + 1) * 512)
                qp = ps.tile([128, 512], F32, tag="pp", bufs=2, name="qp")
                for kk in range(4):
                    nc.tensor.matmul(qp, sb_wq[:, kk, :], xt[:, kk, ns],
                                     start=(kk == 0), stop=(kk == 3))
                nc.vector.tensor_scalar_add(out=q2[:, ns], in0=qp, scalar1=qc)

            def emit_v(n):
                ns = slice(n * 512, (n + 1) * 512)
                vp = ps.tile([128, 512], F32, tag="pp", bufs=2, name="vp")
                for kk in range(4):
                    nc.tensor.matmul(vp, sb_wv[:, kk, :], xt[:, kk, ns],
                                     start=(kk == 0), stop=(kk == 3))
                nc.vector.tensor_scalar_add(out=vc2[:, ns], in0=vp, scalar1=vc_b)
                # XBAR-transpose this 512-col span into 4 vt chunks per head
                cs = slice(4 * n, 4 * n + 4)
                nc.sync.dma_start_transpose(out=vt[:, cs, 0:64], in_=vc2[0:64, ns])
                nc.sync.dma_start_transpose(out=vt[:, cs, 80:144], in_=vc2[64:128, ns])

            for n in range(8):
                ns = slice(n * 512, (n + 1) * 512)
                kp = ps.tile([128, 512], F32, tag="pp", bufs=2, name="kp")
                for kk in range(4):
                    nc.tensor.matmul(kp, sb_wk[:, kk, :], xt[:, kk, ns],
                                     start=(kk == 0), stop=(kk == 3))
                nc.vector.tensor_scalar_add(out=k2[:, ns], in0=kp, scalar1=kc)
            emit_q(0)
            emit_q(1)
            emit_v(0)
            emit_v(1)

            # ---- attention ----
            # Per (h, tsup) stripe of 1024 t-columns.  exp (ACT) is the
            # bottleneck and runs back-to-back; the a_plus accumulation lags
            # one j.  Remaining q/v chunks stream into stripe (0,0)'s slack.
            a_cats = [big.tile([128, TSUP], BF16, name=f"a_cat{i}")
                      for i in range(NT)]

            def recip_neg(den, width):
                """z = -1/den at ~18 bits via NOT-seed + 2 NR steps, all as
                plain DVE ops (the scheduler models these accurately, unlike
                the 8-pass InstReciprocal)."""
                nxz = work.tile([1, width], I32, tag="nx", name="nxz")
                nc.vector.tensor_scalar(out=nxz, in0=den.bitcast(I32),
                                        scalar1=m1c, scalar2=None,
                                        op0=ALU.bitwise_xor)
                # partition-0 copy of den (den may live on partition 64):
                # den0 = NOT(NOT(den)) — tensor_scalar allows cross-partition
                # bases, tensor_tensor below does not
                den0 = work.tile([1, width], F32, tag="rd", name="den0")
                nc.vector.tensor_scalar(out=den0.bitcast(I32), in0=nxz,
                                        scalar1=m1c, scalar2=None,
                                        op0=ALU.bitwise_xor)
                z = work.tile([1, width], F32, tag="rz", name="rz")
                nc.vector.tensor_scalar_mul(out=z, in0=nxz.bitcast(F32),
                                            scalar1=0.23549792)
                u = work.tile([1, width], F32, tag="ru", name="ru")
                nc.vector.tensor_mul(u, den0, z)
                nc.vector.scalar_tensor_tensor(out=z, in0=u, scalar=2.0017324,
                                               in1=z, op0=ALU.add, op1=ALU.mult)
                nc.vector.tensor_mul(u, den0, z)
                nc.vector.scalar_tensor_tensor(out=z, in0=u, scalar=2.0,
                                               in1=z, op0=ALU.add, op1=ALU.mult)
                return z

            def emit_normalize(key, acp_t):
                hh, ts_idx = key
                hsn = slice(CH * hh, CH * (hh + 1))
                den = acp_t[64:65, :, :].rearrange("p a b -> p (a b)")
                z = recip_neg(den, 1024)
                for tg in range(2):
                    tsl = slice(tg * 512, (tg + 1) * 512)
                    rbc = work.tile([64, 512], F32, tag="rbc", name="rbc")
                    nc.gpsimd.partition_broadcast(rbc, z[:, tg * 512:(tg + 1) * 512])
                    nc.vector.scalar_tensor_tensor(
                        out=a_cats[ts_idx][hsn, tsl], in0=acp_t[0:64, tg, :],
                        scalar=-1.0, in1=rbc, op0=ALU.mult, op1=ALU.mult)

            def emit_proj_unit(ts_idx, u, tag="pp", on_act=False):
                # one [128,512] unit of the projection of t-stripe ts_idx.
                # on_act: do the PSUM->SBUF cast on the scalar engine (only
                # sensible in the tail, after the last exp, when ACT is idle)
                m, n = u >> 1, u & 1
                tb = ts_idx * TSUP
                ms = slice(m * 128, (m + 1) * 128)
                pp = ps.tile([128, 512], F32, tag=tag, bufs=2, name="pp")
                nc.tensor.matmul(pp, sb_pw[:, ms],
                                 a_cats[ts_idx][:, n * 512:(n + 1) * 512],
                                 start=True, stop=True)
                pt = work.tile([128, 512], BF16, tag="pt", bufs=4, name="pt")
                if on_act:
                    nc.scalar.activation(out=pt, in_=pp, func=AF.Copy)
                else:
                    nc.vector.tensor_scalar_add(out=pt, in0=pp, scalar1=0.0)
                nc.sync.dma_start(out=part[ms, tb + n * 512:tb + (n + 1) * 512], in_=pt)

            pending_norm = None   # (key, acp) not yet normalized
            for tsup in range(NT):
                t0 = tsup * TSUP
                for h in range(HEADS_PER_CORE):
                    hs = slice(CH * h, CH * (h + 1))
                    vs = slice(80 * h, 80 * h + 65)
                    apl = []
                    for tg in range(2):
                        ap_t = ps.tile([65, 512], F32, tag=f"apl{tg}", name=f"apl{tg}")
                        apl.append(ap_t)
                    prevE = None
                    for j in range(SJ + 1):
                        if j == 1 and pending_norm is not None:
                            emit_normalize(*pending_norm)
                            pending_norm = None
                        if h == 1 and tsup > 0 and 14 <= j < 30 and (j - 14) % 2 == 0:
                            emit_proj_unit(tsup - 1, (j - 14) // 2)
                        if tsup == 0 and h == 0:
                            if j in (3, 7) and j < SJ:
                                emit_q(2 + (j - 3) // 4)
                            if j in (2, 6, 10, 14, 18, 22) and j < SJ:
                                emit_v(2 + (j - 2) // 4)
                        if tsup == 0 and h == 1:
                            if j in (3, 7, 11, 15) and j < SJ:
                                emit_q(4 + (j - 3) // 4)
                        if j < SJ:
                            js = slice(j * 128, (j + 1) * 128)
                            sc = ps.tile([128, 1024], F32, tag="sc", bufs=2, name="sc")
                            nc.tensor.matmul(sc[:, 0:512], k2[hs, js],
                                             q2[hs, t0:t0 + 512], start=True, stop=True)
                            nc.tensor.matmul(sc[:, 512:1024], k2[hs, js],
                                             q2[hs, t0 + 512:t0 + 1024],
                                             start=True, stop=True)
                            E = work.tile([128, 1024], BF16, tag="E",
                                          bufs=8, name="E")
                            nc.scalar.activation(out=E, in_=sc, func=AF.Exp,
                                                 scale=0.125)
                        if prevE is not None:
                            jj = j - 1
                            st = (jj == 0)
                            sp = (jj == SJ - 1)
                            nc.tensor.matmul(apl[0], vt[:, jj, vs],
                                             prevE[:, 0:512], start=st, stop=sp)
                            nc.tensor.matmul(apl[1], vt[:, jj, vs],
                                             prevE[:, 512:1024], start=st, stop=sp)
                        prevE = E if j < SJ else None
                    if tsup == NT - 1 and h == 1:
                        # final stripe: normalize straight out of PSUM and
                        # interleave the last projection per t-half
                        for tg in range(2):
                            tsl = slice(tg * 512, (tg + 1) * 512)
                            # recip on the now-idle ACT engine: 1/d = exp(-ln d)
                            lnd = work.tile([1, 512], F32, tag="lnd", name="lnd")
                            nc.scalar.activation(out=lnd, in_=apl[tg][64:65, :],
                                                 func=AF.Ln)
                            rec = work.tile([1, 512], F32, tag="rec", name="rec")
                            nc.scalar.activation(out=rec, in_=lnd, func=AF.Exp,
                                                 scale=-1.0)
                            rbc = work.tile([64, 512], F32, tag="rbc", name="rbc")
                            nc.gpsimd.partition_broadcast(rbc, rec)
                            nc.vector.tensor_mul(a_cats[NT - 1][hs, tsl],
                                                 apl[tg][0:64, :], rbc)
                            for m in range(4):
                                emit_proj_unit(NT - 1, 2 * m + tg,
                                               tag=("pp", "sc")[m % 2],
                                               on_act=True)
                    else:
                        # move a_plus off PSUM quickly (releases the apl banks)
                        acp = work.tile([65, 2, 512], F32, tag="acp", bufs=3, name="acp")
                        for tg in range(2):
                            nc.vector.tensor_copy(out=acp[:, tg, :], in_=apl[tg])
                        pending_norm = ((h, tsup), acp)

    nc.compile()
    return nc


def get_program():
    global _PROGRAM
    if _PROGRAM is None:
        _PROGRAM = build_program()
    return _PROGRAM


def make_in_maps(x, norm_w, norm_b, qkv_w, qkv_b, proj_w):
    """Build the 8 per-core input maps from full inputs."""
    f = np.float32
    bf = ml_dtypes.bfloat16
    # [B, 128, 4, L]: partition-major so each partition's x DMA is one
    # contiguous 32KB run (channel = t*128 + p)
    x2 = np.ascontiguousarray(
        x.reshape(B, 4, 128, L).transpose(0, 2, 1, 3), dtype=bf)

    gmask = np.zeros((128, 4, G), dtype=f)
    bmask = np.zeros((G, 4, 128), dtype=f)
    for t in range(4):
        for p in range(128):
            g = (t * 128 + p) // 16
            gmask[p, t, g] = 1.0 / 16.0
            bmask[g, t, p] = 1.0
    gamma4 = np.ascontiguousarray(norm_w.reshape(4, 128), dtype=f)
    beta4 = np.ascontiguousarray(norm_b.reshape(4, 128), dtype=f)

    in_maps = []
    for cid in range(N_CORES):
        b = cid // 4
        h0 = 2 * (cid % 4)
        h1 = h0 + 1
        qrows = list(range(192 * h0, 192 * h0 + 64)) + list(range(192 * h1, 192 * h1 + 64))
        krows = [r + 64 for r in qrows]
        vrows = [r + 128 for r in qrows]
        wqT = np.ascontiguousarray(qkv_w[qrows, :].T, dtype=bf)
        wkT = np.ascontiguousarray(qkv_w[krows, :].T, dtype=bf)
        wvT = np.ascontiguousarray(qkv_w[vrows, :].T, dtype=bf)
        qbv = np.ascontiguousarray(qkv_b[qrows], dtype=f)
        kbv = np.ascontiguousarray(qkv_b[krows], dtype=f)
        vbv = np.ascontiguousarray(qkv_b[vrows], dtype=f)
        ch_cols = list(range(64 * h0, 64 * h0 + 64)) + list(range(64 * h1, 64 * h1 + 64))
        pwT = np.ascontiguousarray(proj_w[:, ch_cols].T, dtype=bf)
        in_maps.append({
            "xb": x2[b], "gmask": gmask, "bmask": bmask,
            "gamma4": gamma4, "beta4": beta4,
            "wqT": wqT, "wkT": wkT, "wvT": wvT,
            "qb": qbv, "kb": kbv, "vb": vbv, "pwT": pwT,
        })
    return in_maps


def kernel(x, norm_w, norm_b, qkv_w, qkv_b, proj_w, proj_b, _trace=False):
    x = np.asarray(x, dtype=np.float32)
    in_maps = make_in_maps(x, np.asarray(norm_w), np.asarray(norm_b),
                           np.asarray(qkv_w), np.asarray(qkv_b), np.asarray(proj_w))
    nc = get_program()
    res = run_bass_kernel_spmd(nc, in_maps, list(range(N_CORES)), trace=_trace)
    hout = np.zeros((B, C, L), dtype=np.float32)
    for cid in range(N_CORES):
        hout[cid // 4] += np.asarray(res.results[cid]["part"], dtype=np.float32)
    hout += np.asarray(proj_b, dtype=np.float32)[None, :, None]
    out = x + hout.reshape(x.shape)
    if _trace:
        return out.astype(np.float32), res
    return out.astype(np.float32)
